# revision 1
# baseline (speedup 1.0000x reference)
"""Bilateral filter (7x7, dilation 1) Trainium2 Bass kernel — v6.

Problem: input [2, 18, 1024, 1024] f32.
  filterable = input[:, :8]; params = -(input[:, 8:]**2)
  range coeffs = params[:, :8], sx = params[:, 8], sy = params[:, 9]
  out[c] = sum_taps w * f_c(shifted) / sum_taps w, c < 3
  w = exp(sum_c r_c (fn_c - f_c)^2 + sx dx^2 + sy dy^2), OOB taps masked.

Sharding: data-parallel over (batch, H): 8 cores, each 256 rows of one batch
image (+3 halo rows / cols, sentinel-padded host-side; 4-col left pad).

Design (measured on HW, per-op):
  * fp16 heavy path: DVE tensor_tensor runs in 2x_1p mode for 2-byte dtypes
    (any element offset), so sub/rmul/tree adds all stream 2 elem/cycle.
  * GPSIMD is completely idle: ANY concurrent Pool-engine op contends for
    the shared SBUF port and slows every DVE op ~5x (probe-verified).
  * channel-planar F tiles [128 rows, 9 ch, 520 cols]: ch 0 = ones plane,
    ch 1..8 = filterable. No per-pixel interleave copies; row shifts come
    from 7 row-shifted DMA loads, col shifts are free-axis views.
  * per tap: DVE sub -> ACT Square -> DVE r*d2 -> DVE 3-level pair-tree ->
    DVE +Asp -> ACT exp -> DVE t4 = w * [1, f0, f1, f2] (ones channel folds
    wsum into the numerator accumulate) -> fp16 chain-add, fp32 acc4 update
    once per 8-tap group.
  * finite sentinel 240.0: d^2 = 57600 stays finite in fp16; r*d^2
    overflows to -inf only via genuinely large products, exp -> +0, and
    -0 * d2 = -0 (no 0*inf NaN path).
  * center tap folded into the accumulator init (w=1).
"""

import sys

if "/opt/trn_rl_repo" not in sys.path:
    sys.path.insert(0, "/opt/trn_rl_repo")

import numpy as np

import concourse.bass as bass
import concourse.mybir as mybir
from concourse.bacc import Bacc
from concourse.tile import TileContext

FP32 = mybir.dt.float32
F16 = mybir.dt.float16

B, C_ALL, H, W = 2, 18, 1024, 1024
CF = 8                      # filterable channels
CO = 3                      # output channels
KS, RAD = 7, 3
HC = H * B // 8             # 256 output rows per core
HIN = HC + 2 * RAD          # 262 input rows per core (halo padded host-side)
LPAD = 4                    # left col pad (4B alignment for bf16 2x mode)
WIN = W + LPAD + RAD + 1    # 1032 input cols per core (halo padded host-side)
WC = 512                    # W chunk
NW = W // WC                # 2
NHB = HC // 128             # 2
WT = WC + LPAD + RAD + 1    # 520 = chunk + col halo (even)
SENT = 240.0                # sentinel: large enough that exp(r*d^2) == 0
D2IDX = {0: 3, 1: 2, 2: 1, 3: 0, 4: 1, 5: 2, 6: 3}   # |k-3| -> index trick
D2VALS = [0.0, 1.0, 4.0, 9.0]
IDX4 = [3, 2, 1, 0, 1, 2, 3]                          # (k-3)^2 class index

_CACHED = {}
TAP_SET = None   # optional [(i,j)] subset for debugging


def build_nc(macros=None):
    nc = Bacc()
    x = nc.dram_tensor("x", [C_ALL, HIN, WIN], FP32, kind="ExternalInput")
    y = nc.dram_tensor("y", [CO, HC, W], FP32, kind="ExternalOutput")

    if macros is None:
        macros = [(hb, wck) for hb in range(NHB) for wck in range(NW)]
    with TileContext(nc) as tc:
        with (
            tc.tile_pool(name="fpool", bufs=1) as fpool,
            tc.tile_pool(name="stpool", bufs=2) as stpool,
            tc.tile_pool(name="cpool", bufs=1) as cpool,
            tc.tile_pool(name="dpool", bufs=2) as dpool,
            tc.tile_pool(name="spool", bufs=3) as spool,
        ):
            for hb, wcki in macros:
                _macro(nc, tc, x, y, fpool, stpool, cpool, dpool, spool,
                       hb, wcki)
    nc.compile()
    return nc


def _macro(nc, tc, x, y, fpool, stpool, cpool, dpool, spool, hb, wck):
    w0 = wck * WC
    r0 = hb * 128

    # ---- load + convert the 7 row-shifted planar F tile sets (bf16) ----
    # channel layout: [ones, f0..f7] (9 planes).  ones-plane lets the
    # numerator fold wsum in as channel 0 of a 4-channel multiply.
    CP = CF + 1
    # param DMAs first: small, lets R/Asp prep overlap F loads
    pst = stpool.tile([128, CF * WC], FP32, tag="pstage", bufs=1,
                      name=f"pst_{hb}_{wck}")
    pst3 = pst[:].rearrange("p (c x) -> p c x", x=WC)
    for c in range(CF):
        nc.sync.dma_start(
            out=pst3[:, c, :],
            in_=x[CF + c, r0 + RAD : r0 + RAD + 128, w0 + LPAD : w0 + LPAD + WC])
    sst = stpool.tile([128, 2 * WC], FP32, tag="sstage", bufs=1,
                      name=f"sst_{hb}_{wck}")
    for k in range(2):
        nc.sync.dma_start(
            out=sst[:, k * WC : (k + 1) * WC],
            in_=x[2 * CF + k, r0 + RAD : r0 + RAD + 128,
                  w0 + LPAD : w0 + LPAD + WC])
    F = [None] * KS  # F[oy]: [128, CP, WT] f16, rows r0+oy .. (slab)
    for oy in (RAD, 0, 1, 2, 4, 5, 6):
        Fi = fpool.tile([128, CP * WT], F16, tag=f"F{oy}", bufs=1,
                        name=f"F{oy}_{hb}_{wck}")
        Fi3 = Fi[:].rearrange("p (c x) -> p c x", x=WT)
        nc.vector.memset(Fi[:, 0:WT], 1.0)
        for c in range(CF):
            st = stpool.tile([128, WT], FP32, tag="stage", bufs=2,
                             name=f"st_{hb}_{wck}_{oy}_{c}")
            nc.sync.dma_start(
                out=st[:],
                in_=x[c, r0 + oy : r0 + oy + 128, w0 : w0 + WT],
            )
            nc.scalar.copy(Fi3[:, 1 + c, :], st[:])
        F[oy] = Fi

    def f3d(oy):
        return F[oy][:].rearrange("p (c x) -> p c x", x=WT)

    Fc = f3d(RAD)[:, 1:CP, LPAD : LPAD + WC]     # center view [128, 8, WC]

    # ---- params: R = -(p*p) fp16 planar, sx2, sy2, Asp ----
    R = cpool.tile([128, CF * WC], F16, tag="R", name=f"R_{hb}_{wck}")
    nc.vector.scalar_tensor_tensor(
        R[:], pst[:], -1.0, pst[:], mybir.AluOpType.mult, mybir.AluOpType.mult)

    sxy = cpool.tile([128, 2 * WC], F16, tag="sxy", name=f"sxy_{hb}_{wck}")
    nc.vector.scalar_tensor_tensor(
        sxy[:], sst[:], -1.0, sst[:], mybir.AluOpType.mult,
        mybir.AluOpType.mult)
    sx2 = sxy[:, 0:WC]
    sy2 = sxy[:, WC : 2 * WC]

    # Asp[ai*4+bi] = A*sx2 + B*sy2 for A,B in {0,1,4,9}
    Asp = cpool.tile([128, 16 * WC], F16, tag="Asp", name=f"Asp_{hb}_{wck}")
    for bi, bval in enumerate(D2VALS):
        syb = spool.tile([128, WC], F16, tag="syb", bufs=2,
                         name=f"syb_{hb}_{wck}_{bi}")
        nc.vector.tensor_scalar_mul(syb[:], sy2, float(bval))
        for ai, aval in enumerate(D2VALS):
            nc.vector.scalar_tensor_tensor(
                Asp[:, (ai * 4 + bi) * WC : (ai * 4 + bi + 1) * WC],
                sx2, float(aval), syb[:],
                mybir.AluOpType.mult, mybir.AluOpType.add)

    # ---- accumulator acc4 = [wsum, acc0, acc1, acc2]; center tap (w=1)
    # folded in via init from [ones, f0, f1, f2] ----
    acc4 = cpool.tile([128, 4 * WC], FP32, tag="acc4", name=f"acc4_{hb}_{wck}")
    nc.scalar.copy(
        acc4[:].rearrange("p (c x) -> p c x", x=WC),
        f3d(RAD)[:, 0:4, LPAD : LPAD + WC])

    # ---- 48 off-center taps, in 6 groups of 8 (bf16 tap-tree accum) ----
    taps = TAP_SET if TAP_SET is not None else [
        (i, j) for i in range(KS) for j in range(KS) if (i, j) != (RAD, RAD)]
    for g0 in range(0, len(taps), 8):
        group = taps[g0 : g0 + 8]
        t4s = []
        for i, j in group:       # oy = i - 3, ox = j - 3
            sh = f3d(i)[:, 1:CP, j + 1 : j + 1 + WC]  # shifted view
            d = dpool.tile([128, CF * WC], F16, tag="d",
                           name=f"d_{hb}_{wck}_{i}_{j}")
            nc.vector.tensor_sub(
                d[:].rearrange("p (c x) -> p c x", x=WC), sh, Fc)
            d2 = d
            nc.scalar.activation(d2[:], d[:],
                                 mybir.ActivationFunctionType.Square)
            rd = dpool.tile([128, CF * WC], F16, tag="rd",
                            name=f"rd_{hb}_{wck}_{i}_{j}")
            nc.vector.tensor_mul(rd[:], R[:], d2[:])
            rd3 = rd[:].rearrange("p (c x) -> p c x", x=WC)
            t1 = spool.tile([128, 4 * WC], F16, tag="t1", bufs=2,
                            name=f"t1_{hb}_{wck}_{i}_{j}")
            nc.vector.tensor_add(
                t1[:].rearrange("p (c x) -> p c x", x=WC),
                rd3[:, 0:4, :], rd3[:, 4:8, :])
            t13 = t1[:].rearrange("p (c x) -> p c x", x=WC)
            t2 = spool.tile([128, 2 * WC], F16, tag="t2", bufs=2,
                            name=f"t2_{hb}_{wck}_{i}_{j}")
            nc.vector.tensor_add(
                t2[:].rearrange("p (c x) -> p c x", x=WC),
                t13[:, 0:2, :], t13[:, 2:4, :])
            st_ = spool.tile([128, WC], F16, tag="s",
                             name=f"s_{hb}_{wck}_{i}_{j}")
            k16 = (IDX4[j] * 4 + IDX4[i]) * WC
            nc.vector.tensor_add(st_[:], t2[:, 0:WC], t2[:, WC : 2 * WC])
            stt = spool.tile([128, WC], F16, tag="s",
                             name=f"stt_{hb}_{wck}_{i}_{j}")
            nc.vector.tensor_add(stt[:], st_[:], Asp[:, k16 : k16 + WC])
            w_t = spool.tile([128, WC], F16, tag="w",
                             name=f"w_{hb}_{wck}_{i}_{j}")
            nc.scalar.activation(w_t[:], stt[:],
                                 mybir.ActivationFunctionType.Exp)
            # numerator+wsum in one: t4 = w * [1, f0, f1, f2]
            t4 = spool.tile([128, 4 * WC], F16, tag="t4", bufs=3,
                            name=f"t4_{hb}_{wck}_{i}_{j}")
            w_b = w_t[:].unsqueeze(1).broadcast_to([128, 4, WC])
            nc.vector.tensor_mul(
                t4[:].rearrange("p (c x) -> p c x", x=WC), w_b,
                f3d(i)[:, 0:4, j + 1 : j + 1 + WC])
            t4s.append(t4)
            # interleave bf16 pair-tree adds so the t4/u rings stay shallow
            while len(t4s) >= 2 and len(t4s) % 2 == 0:
                u = spool.tile([128, 4 * WC], F16, tag="u", bufs=3,
                               name=f"u_{hb}_{wck}_{i}_{j}_{len(t4s)}")
                nc.vector.tensor_add(u[:], t4s[-2][:], t4s[-1][:])
                t4s = t4s[:-2] + [u]
        while len(t4s) > 1:
            u = spool.tile([128, 4 * WC], F16, tag="u", bufs=3,
                           name=f"ru_{hb}_{wck}_{g0}_{len(t4s)}")
            nc.vector.tensor_add(u[:], t4s[-2][:], t4s[-1][:])
            t4s = t4s[:-2] + [u]
        nc.vector.tensor_add(acc4[:], acc4[:], t4s[0][:])

    # ---- out = acc / wsum ----
    rec = spool.tile([128, WC], FP32, tag="rec", bufs=1,
                     name=f"rec_{hb}_{wck}")
    nc.vector.reciprocal(rec[:], acc4[:, 0:WC])
    out3 = spool.tile([128, CO * WC], FP32, tag="out3", bufs=1,
                      name=f"out3_{hb}_{wck}")
    rec_b = rec[:].unsqueeze(1).broadcast_to([128, CO, WC])
    nc.vector.tensor_mul(
        out3[:].rearrange("p (c x) -> p c x", x=WC), rec_b,
        acc4[:].rearrange("p (c x) -> p c x", x=WC)[:, 1:4, :])
    o3 = out3[:].rearrange("p (c x) -> p c x", x=WC)
    for c in range(CO):
        nc.sync.dma_start(out=y[c, r0 : r0 + 128, w0 : w0 + WC],
                          in_=o3[:, c, :])


def shard_inputs(input):
    """input [2,18,1024,1024] -> 8 per-core slabs [18, 262, 1030]."""
    input = np.asarray(input, dtype=np.float32)
    per_b = 4
    rows = H // per_b
    in_maps = []
    for core in range(8):
        b, q = divmod(core, per_b)
        r0 = q * rows
        slab = np.full((C_ALL, HIN, WIN), SENT, dtype=np.float32)
        s_lo = max(r0 - RAD, 0)
        s_hi = min(r0 + rows + RAD, H)
        slab[:, s_lo - (r0 - RAD) : s_hi - (r0 - RAD), LPAD : LPAD + W] = \
            input[b, :, s_lo:s_hi, :]
        in_maps.append({"x": np.ascontiguousarray(slab)})
    return in_maps


def assemble(results):
    out = np.empty((B, CO, H, W), dtype=np.float32)
    rows = H // 4
    for core in range(8):
        b, q = divmod(core, 4)
        out[b, :, q * rows : (q + 1) * rows, :] = results[core]["y"]
    return out


def kernel(input):
    from concourse.bass_utils import run_bass_kernel_spmd

    if "nc" not in _CACHED:
        _CACHED["nc"] = build_nc()
    in_maps = shard_inputs(input)
    res = run_bass_kernel_spmd(_CACHED["nc"], in_maps, list(range(8)))
    return assemble(res.results)



# revision 2
# speedup vs baseline: 1.5865x; 1.5865x over previous
"""Bilateral filter (7x7, dilation 1) Trainium2 Bass kernel — v7.

Problem: input [2, 18, 1024, 1024] f32.
  filterable = input[:, :8]; params = -(input[:, 8:]**2)
  range coeffs = params[:, :8], sx = params[:, 8], sy = params[:, 9]
  out[c] = sum_taps w * f_c(shifted) / sum_taps w, c < 3
  w = exp(sum_c r_c (fn_c - f_c)^2 + sx dx^2 + sy dy^2), OOB taps masked.

Sharding: data-parallel over (batch, H): 8 cores, each 256 rows of one batch
image (+3 halo rows / cols, sentinel-padded host-side; 4-col left pad).

v7 design (v6 measured: DVE 94.8% busy was the wall; ACT 50%, PE idle):
  * per tap: DVE sub -> ACT Square -> DVE r*d2 -> PE: 8 identity-stationary
    matmuls accumulate the channel slabs + 1 matmul adds Asp, all into a
    PSUM bank => logw in fp32 PSUM (the old DVE add-tree is gone).
  * ACT Exp reads the PSUM bank directly -> w (f16, SBUF).
  * DVE t3 = w * [f0,f1,f2]; PE identity matmuls accumulate [w|t3] into 4
    persistent PSUM banks (acc) across all 49 taps (old fp16 tap-tree and
    fp32 acc adds gone). Center tap = the start=True init matmuls (w=1).
  * DVE per tap: sub(8ch) + rmul(8ch) + t3(3ch) only.
  * sentinel 24.0 (was 240): keeps r*d^2 finite in fp16 so the identity
    matmul never sees -inf (0 * -inf = NaN through the PE). exp(~-340*p2
    sum) == 0 still masks OOB taps.
  * PSUM budget: 4 acc banks + 3 rotating logw banks = 7 of 8.
"""

import sys

if "/opt/trn_rl_repo" not in sys.path:
    sys.path.insert(0, "/opt/trn_rl_repo")

import numpy as np

import concourse.bass as bass
import concourse.mybir as mybir
from concourse.bacc import Bacc
from concourse.masks import make_identity
from concourse.tile import TileContext

FP32 = mybir.dt.float32
F16 = mybir.dt.float16

B, C_ALL, H, W = 2, 18, 1024, 1024
CF = 8                      # filterable channels
CO = 3                      # output channels
KS, RAD = 7, 3
HC = H * B // 8             # 256 output rows per core
HIN = HC + 2 * RAD          # 262 input rows per core (halo padded host-side)
LPAD = 4                    # left col pad (4B alignment for f16 2x mode)
WIN = W + LPAD + RAD + 1    # 1032 input cols per core (halo padded host-side)
WC = 512                    # W chunk (= one PSUM bank of fp32)
NW = W // WC                # 2
NHB = HC // 128             # 2
WT = WC + LPAD + RAD + 1    # 520 = chunk + col halo (even)
SENT = 24.0                 # sentinel: exp(r*d^2) == 0, r*d^2 finite in fp16
D2VALS = [0.0, 1.0, 4.0, 9.0]
IDX4 = [3, 2, 1, 0, 1, 2, 3]                          # (k-3)^2 class index

_CACHED = {}
TAP_SET = None   # optional [(i,j)] subset for debugging


def build_nc(macros=None):
    nc = Bacc()
    x = nc.dram_tensor("x", [C_ALL, HIN, WIN], FP32, kind="ExternalInput")
    y = nc.dram_tensor("y", [CO, HC, W], FP32, kind="ExternalOutput")

    if macros is None:
        macros = [(hb, wck) for hb in range(NHB) for wck in range(NW)]
    with TileContext(nc) as tc:
        with (
            tc.tile_pool(name="gpool", bufs=1) as gpool,
            tc.tile_pool(name="fpool", bufs=1) as fpool,
            tc.tile_pool(name="stpool", bufs=2) as stpool,
            tc.tile_pool(name="cpool", bufs=1) as cpool,
            tc.tile_pool(name="dpool", bufs=2) as dpool,
            tc.tile_pool(name="spool", bufs=3) as spool,
            tc.psum_pool(name="papool", bufs=1) as papool,
            tc.psum_pool(name="plpool", bufs=3) as plpool,
        ):
            ident = gpool.tile([128, 128], F16, tag="ident", name="ident")
            make_identity(nc, ident[:])
            for hb, wcki in macros:
                _macro(nc, tc, x, y, ident, fpool, stpool, cpool, dpool,
                       spool, papool, plpool, hb, wcki)
    nc.compile()
    return nc


def _macro(nc, tc, x, y, ident, fpool, stpool, cpool, dpool, spool,
           papool, plpool, hb, wck):
    w0 = wck * WC
    r0 = hb * 128

    # ---- load + convert the 7 row-shifted planar F tile sets (f16) ----
    # param DMAs first: small, lets R/Asp prep overlap F loads
    pst = stpool.tile([128, CF * WC], FP32, tag="pstage", bufs=1,
                      name=f"pst_{hb}_{wck}")
    pst3 = pst[:].rearrange("p (c x) -> p c x", x=WC)
    for c in range(CF):
        nc.sync.dma_start(
            out=pst3[:, c, :],
            in_=x[CF + c, r0 + RAD : r0 + RAD + 128, w0 + LPAD : w0 + LPAD + WC])
    sst = stpool.tile([128, 2 * WC], FP32, tag="sstage", bufs=1,
                      name=f"sst_{hb}_{wck}")
    for k in range(2):
        nc.sync.dma_start(
            out=sst[:, k * WC : (k + 1) * WC],
            in_=x[2 * CF + k, r0 + RAD : r0 + RAD + 128,
                  w0 + LPAD : w0 + LPAD + WC])
    F = [None] * KS  # F[oy]: [128, CF, WT] f16, rows r0+oy .. (slab)
    for oy in (RAD, 0, 1, 2, 4, 5, 6):
        Fi = fpool.tile([128, CF * WT], F16, tag=f"F{oy}", bufs=1,
                        name=f"F{oy}_{hb}_{wck}")
        Fi3 = Fi[:].rearrange("p (c x) -> p c x", x=WT)
        for c in range(CF):
            st = stpool.tile([128, WT], FP32, tag="stage", bufs=2,
                             name=f"st_{hb}_{wck}_{oy}_{c}")
            nc.sync.dma_start(
                out=st[:],
                in_=x[c, r0 + oy : r0 + oy + 128, w0 : w0 + WT],
            )
            nc.scalar.copy(Fi3[:, c, :], st[:])
        F[oy] = Fi

    def f3d(oy):
        return F[oy][:].rearrange("p (c x) -> p c x", x=WT)

    Fc = f3d(RAD)[:, :, LPAD : LPAD + WC]     # center view [128, 8, WC]

    # ---- params: R = -(p*p) f16 planar, sx2, sy2, Asp ----
    R = cpool.tile([128, CF * WC], F16, tag="R", name=f"R_{hb}_{wck}")
    nc.vector.scalar_tensor_tensor(
        R[:], pst[:], -1.0, pst[:], mybir.AluOpType.mult, mybir.AluOpType.mult)

    sxy = cpool.tile([128, 2 * WC], F16, tag="sxy", name=f"sxy_{hb}_{wck}")
    nc.vector.scalar_tensor_tensor(
        sxy[:], sst[:], -1.0, sst[:], mybir.AluOpType.mult,
        mybir.AluOpType.mult)
    sx2 = sxy[:, 0:WC]
    sy2 = sxy[:, WC : 2 * WC]

    # Asp[ai*4+bi] = A*sx2 + B*sy2 for A,B in {0,1,4,9}
    Asp = cpool.tile([128, 16 * WC], F16, tag="Asp", name=f"Asp_{hb}_{wck}")
    for bi, bval in enumerate(D2VALS):
        syb = spool.tile([128, WC], F16, tag="syb", bufs=2,
                         name=f"syb_{hb}_{wck}_{bi}")
        nc.vector.tensor_scalar_mul(syb[:], sy2, float(bval))
        for ai, aval in enumerate(D2VALS):
            nc.vector.scalar_tensor_tensor(
                Asp[:, (ai * 4 + bi) * WC : (ai * 4 + bi + 1) * WC],
                sx2, float(aval), syb[:],
                mybir.AluOpType.mult, mybir.AluOpType.add)

    # ---- persistent PSUM accumulators: wsum + 3 out channels ----
    # center tap (w=1) folded into the start=True init matmuls.
    ones = cpool.tile([128, WC], F16, tag="ones", name=f"ones_{hb}_{wck}")
    nc.vector.memset(ones[:], 1.0)
    accW = papool.tile([128, WC], FP32, tag="accW", name=f"accW_{hb}_{wck}")
    accC = [papool.tile([128, WC], FP32, tag=f"acc{c}",
                        name=f"acc{c}_{hb}_{wck}") for c in range(CO)]
    nc.tensor.matmul(out=accW[:], lhsT=ident[:], rhs=ones[:],
                     start=True, stop=False)
    for c in range(CO):
        nc.tensor.matmul(out=accC[c][:], lhsT=ident[:], rhs=Fc[:, c, :],
                         start=True, stop=False)

    # ---- 48 off-center taps ----
    taps = TAP_SET if TAP_SET is not None else [
        (i, j) for i in range(KS) for j in range(KS) if (i, j) != (RAD, RAD)]
    n_taps = len(taps)
    for ti, (i, j) in enumerate(taps):   # oy = i - 3, ox = j - 3
        last = ti == n_taps - 1
        sh = f3d(i)[:, :, j + 1 : j + 1 + WC]  # shifted view [128, 8, WC]
        d = dpool.tile([128, CF * WC], F16, tag="d",
                       name=f"d_{hb}_{wck}_{i}_{j}")
        nc.vector.tensor_sub(
            d[:].rearrange("p (c x) -> p c x", x=WC), sh, Fc)
        d2 = d
        nc.scalar.activation(d2[:], d[:],
                             mybir.ActivationFunctionType.Square)
        rd = dpool.tile([128, CF * WC], F16, tag="rd",
                        name=f"rd_{hb}_{wck}_{i}_{j}")
        nc.vector.tensor_mul(rd[:], R[:], d2[:])
        rd3 = rd[:].rearrange("p (c x) -> p c x", x=WC)
        # PE: channel reduce + Asp add, accumulated in a PSUM bank
        logw = plpool.tile([128, WC], FP32, tag="logw", bufs=3,
                           name=f"logw_{hb}_{wck}_{i}_{j}")
        for c in range(CF):
            nc.tensor.matmul(out=logw[:], lhsT=ident[:], rhs=rd3[:, c, :],
                             start=(c == 0), stop=False)
        k16 = (IDX4[j] * 4 + IDX4[i]) * WC
        nc.tensor.matmul(out=logw[:], lhsT=ident[:],
                         rhs=Asp[:, k16 : k16 + WC], start=False, stop=True)
        w_t = spool.tile([128, WC], F16, tag="w",
                         name=f"w_{hb}_{wck}_{i}_{j}")
        nc.scalar.activation(w_t[:], logw[:],
                             mybir.ActivationFunctionType.Exp)
        # numerator: t3 = w * [f0, f1, f2]
        t3 = spool.tile([128, CO * WC], F16, tag="t3", bufs=3,
                        name=f"t3_{hb}_{wck}_{i}_{j}")
        w_b = w_t[:].unsqueeze(1).broadcast_to([128, CO, WC])
        nc.vector.tensor_mul(
            t3[:].rearrange("p (c x) -> p c x", x=WC), w_b,
            f3d(i)[:, 0:CO, j + 1 : j + 1 + WC])
        t33 = t3[:].rearrange("p (c x) -> p c x", x=WC)
        nc.tensor.matmul(out=accW[:], lhsT=ident[:], rhs=w_t[:],
                         start=False, stop=last)
        for c in range(CO):
            nc.tensor.matmul(out=accC[c][:], lhsT=ident[:], rhs=t33[:, c, :],
                             start=False, stop=last)

    # ---- out = acc / wsum ----
    rec = spool.tile([128, WC], FP32, tag="rec", bufs=1,
                     name=f"rec_{hb}_{wck}")
    nc.vector.reciprocal(rec[:], accW[:])
    out3 = spool.tile([128, CO * WC], FP32, tag="out3", bufs=1,
                      name=f"out3_{hb}_{wck}")
    for c in range(CO):
        nc.vector.tensor_mul(out3[:, c * WC : (c + 1) * WC], rec[:],
                             accC[c][:])
    o3 = out3[:].rearrange("p (c x) -> p c x", x=WC)
    for c in range(CO):
        nc.sync.dma_start(out=y[c, r0 : r0 + 128, w0 : w0 + WC],
                          in_=o3[:, c, :])


def shard_inputs(input):
    """input [2,18,1024,1024] -> 8 per-core slabs [18, 262, 1030]."""
    input = np.asarray(input, dtype=np.float32)
    per_b = 4
    rows = H // per_b
    in_maps = []
    for core in range(8):
        b, q = divmod(core, per_b)
        r0 = q * rows
        slab = np.full((C_ALL, HIN, WIN), SENT, dtype=np.float32)
        s_lo = max(r0 - RAD, 0)
        s_hi = min(r0 + rows + RAD, H)
        slab[:, s_lo - (r0 - RAD) : s_hi - (r0 - RAD), LPAD : LPAD + W] = \
            input[b, :, s_lo:s_hi, :]
        in_maps.append({"x": np.ascontiguousarray(slab)})
    return in_maps


def assemble(results):
    out = np.empty((B, CO, H, W), dtype=np.float32)
    rows = H // 4
    for core in range(8):
        b, q = divmod(core, 4)
        out[b, :, q * rows : (q + 1) * rows, :] = results[core]["y"]
    return out


def kernel(input):
    from concourse.bass_utils import run_bass_kernel_spmd

    if "nc" not in _CACHED:
        _CACHED["nc"] = build_nc()
    in_maps = shard_inputs(input)
    res = run_bass_kernel_spmd(_CACHED["nc"], in_maps, list(range(8)))
    return assemble(res.results)


# revision 5
# speedup vs baseline: 1.6337x; 1.0298x over previous
"""Bilateral filter (7x7, dilation 1) Trainium2 Bass kernel — v8.

Problem: input [2, 18, 1024, 1024] f32.
  filterable = input[:, :8]; params = -(input[:, 8:]**2)
  range coeffs = params[:, :8], sx = params[:, 8], sy = params[:, 9]
  out[c] = sum_taps w * f_c(shifted) / sum_taps w, c < 3
  w = exp(sum_c r_c (fn_c - f_c)^2 + sx dx^2 + sy dy^2), OOB taps masked.

Sharding: data-parallel over (batch, H): 8 cores, each 256 rows of one batch
image (+3 halo rows / cols, sentinel-padded host-side; 4-col left pad).

v8 design (v7 measured 1241us: DVE 1113us busy, ACT 958, PE 728):
  * per tap: DVE sub -> ACT Square -> DVE rd+ = R+ * d2 -> PE: 8 neg-identity
    matmuls + 1 Asp+ matmul accumulate -logw in a PSUM bank -> ACT Exp reads
    PSUM -> DVE t3 = w * [f0,f1,f2] -> PE +identity matmuls accumulate w and
    t3 into persistent PSUM acc banks across all 49 taps.
  * all-positive moving data (R+ = p^2 via one ACT Square, Asp+ = a*sx2+b*sy2
    via 4x tensor_scalar + 2x tensor_add), sign flip lives in the -I
    stationary: kills the 1x-mode scalar_tensor_tensor prep of v6/v7.
  * F tiles: one fp32->f16 conversion per 134-row slab (ACT, 2 big copies),
    then the 7 row-shifted working tiles are partition-offset SBUF->SBUF
    DMA copies (FA16 + 6-row FB16 tail); F[0] aliases FA16. Replaces 56
    per-macro ACT conversions and 4x the HBM traffic.
  * sentinel 24.0: r*d^2 finite in fp16 (no -inf: 0 * -inf = NaN in the PE).
  * PSUM: accW 1 + accC 3 + logw rotation 4 = 8 banks.
"""

import sys

if "/opt/trn_rl_repo" not in sys.path:
    sys.path.insert(0, "/opt/trn_rl_repo")

import numpy as np

import concourse.bass as bass
import concourse.mybir as mybir
from concourse.bacc import Bacc
from concourse.masks import make_identity
from concourse.tile import TileContext

FP32 = mybir.dt.float32
F16 = mybir.dt.float16

B, C_ALL, H, W = 2, 18, 1024, 1024
CF = 8                      # filterable channels
CO = 3                      # output channels
KS, RAD = 7, 3
HC = H * B // 8             # 256 output rows per core
HIN = HC + 2 * RAD          # 262 input rows per core (halo padded host-side)
LPAD = 4                    # left col pad (4B alignment for f16 2x mode)
WIN = W + LPAD + RAD + 1    # 1032 input cols per core (halo padded host-side)
WC = 512                    # W chunk (= one PSUM bank of fp32)
NW = W // WC                # 2
NHB = HC // 128             # 2
WT = WC + LPAD + RAD + 1    # 520 = chunk + col halo (even)
SENT = 24.0                 # sentinel: exp(-p2*d^2 sum) == 0, finite in fp16
IDX4 = [3, 2, 1, 0, 1, 2, 3]                          # (k-3)^2 class index

_CACHED = {}
TAP_SET = None   # optional [(i,j)] subset for debugging


def build_nc(macros=None):
    nc = Bacc()
    x = nc.dram_tensor("x", [C_ALL, HIN, WIN], FP32, kind="ExternalInput")
    y = nc.dram_tensor("y", [CO, HC, W], FP32, kind="ExternalOutput")

    if macros is None:
        macros = [(hb, wck) for hb in range(NHB) for wck in range(NW)]
    with TileContext(nc) as tc:
        with (
            tc.tile_pool(name="gpool", bufs=1) as gpool,
            tc.tile_pool(name="fpool", bufs=1) as fpool,
            tc.tile_pool(name="stpool", bufs=2) as stpool,
            tc.tile_pool(name="cpool", bufs=1) as cpool,
            tc.tile_pool(name="dpool", bufs=2) as dpool,
            tc.tile_pool(name="spool", bufs=3) as spool,
            tc.psum_pool(name="papool", bufs=1) as papool,
            tc.psum_pool(name="plpool", bufs=4) as plpool,
        ):
            ident = gpool.tile([128, 128], F16, tag="ident", name="ident")
            make_identity(nc, ident[:])
            identN = gpool.tile([128, 128], F16, tag="identN", name="identN")
            nc.vector.tensor_scalar_mul(identN[:], ident[:], -1.0)
            for hb, wcki in macros:
                _macro(nc, tc, x, y, ident, identN, fpool, stpool, cpool,
                       dpool, spool, papool, plpool, hb, wcki)
    nc.compile()
    return nc


def _macro(nc, tc, x, y, ident, identN, fpool, stpool, cpool, dpool, spool,
           papool, plpool, hb, wck):
    w0 = wck * WC
    r0 = hb * 128

    # ---- param DMAs first: small, lets R/Asp prep overlap F loads ----
    pst = stpool.tile([128, CF * WC], FP32, tag="pstage", bufs=1,
                      name=f"pst_{hb}_{wck}")
    pst3 = pst[:].rearrange("p (c x) -> p c x", x=WC)
    for c in range(CF):
        nc.sync.dma_start(
            out=pst3[:, c, :],
            in_=x[CF + c, r0 + RAD : r0 + RAD + 128, w0 + LPAD : w0 + LPAD + WC])
    sst = stpool.tile([128, 2 * WC], FP32, tag="sstage", bufs=1,
                      name=f"sst_{hb}_{wck}")
    for k in range(2):
        nc.sync.dma_start(
            out=sst[:, k * WC : (k + 1) * WC],
            in_=x[2 * CF + k, r0 + RAD : r0 + RAD + 128,
                  w0 + LPAD : w0 + LPAD + WC])

    # ---- F tiles: convert the 134-row slab once, DMA-shift 7 views ----
    FA16 = fpool.tile([128, CF * WT], F16, tag="FA16", bufs=1,
                      name=f"FA16_{hb}_{wck}")
    FA3 = FA16[:].rearrange("p (c x) -> p c x", x=WT)
    for half in range(2):
        st = stpool.tile([128, 4 * WT], FP32, tag="stage", bufs=2,
                         name=f"stA_{hb}_{wck}_{half}")
        st3 = st[:].rearrange("p (c x) -> p c x", x=WT)
        for ci in range(4):
            c = half * 4 + ci
            nc.sync.dma_start(
                out=st3[:, ci, :],
                in_=x[c, r0 : r0 + 128, w0 : w0 + WT])
        nc.scalar.copy(FA3[:, half * 4 : half * 4 + 4, :], st3[:])
    FB16 = fpool.tile([2 * RAD, CF * WT], F16, tag="FB16", bufs=1,
                      name=f"FB16_{hb}_{wck}")
    FB3 = FB16[:].rearrange("p (c x) -> p c x", x=WT)
    stB = stpool.tile([2 * RAD, CF * WT], FP32, tag="stageB", bufs=1,
                      name=f"stB_{hb}_{wck}")
    stB3 = stB[:].rearrange("p (c x) -> p c x", x=WT)
    for c in range(CF):
        nc.sync.dma_start(
            out=stB3[:, c, :],
            in_=x[c, r0 + 128 : r0 + 128 + 2 * RAD, w0 : w0 + WT])
    nc.scalar.copy(FB16[:], stB[:])

    F = [None] * KS  # F[oy]: [128, CF, WT] f16, rows r0+oy .. r0+oy+127
    F[0] = FA16
    for oy in (RAD, 1, 2, 4, 5, 6):
        Fi = fpool.tile([128, CF * WT], F16, tag=f"F{oy}", bufs=1,
                        name=f"F{oy}_{hb}_{wck}")
        nc.sync.dma_start(out=Fi[0 : 128 - oy, :], in_=FA16[oy : 128, :])
        nc.sync.dma_start(out=Fi[128 - oy : 128, :], in_=FB16[0 : oy, :])
        F[oy] = Fi

    def f3d(oy):
        return F[oy][:].rearrange("p (c x) -> p c x", x=WT)

    Fc = f3d(RAD)[:, :, LPAD : LPAD + WC]     # center view [128, 8, WC]

    # ---- params (all positive; sign lives in the -I stationary) ----
    # R+ = p^2 (one ACT Square), sx2 = sx_raw^2, sy2 = sy_raw^2
    R = cpool.tile([128, CF * WC], F16, tag="R", name=f"R_{hb}_{wck}")
    nc.scalar.activation(R[:], pst[:], mybir.ActivationFunctionType.Square)
    sxy = cpool.tile([128, 2 * WC], F16, tag="sxy", name=f"sxy_{hb}_{wck}")
    nc.scalar.activation(sxy[:], sst[:], mybir.ActivationFunctionType.Square)
    sx2 = sxy[:, 0:WC]
    sy2 = sxy[:, WC : 2 * WC]

    # Asp+[(ai,bi)] = A*sx2 + B*sy2, A,B in {0,1,4,9}; (0,0) never used.
    ax = {1: sx2, 2: None, 3: None}
    by = {1: sy2, 2: None, 3: None}
    for k, val in ((2, 4.0), (3, 9.0)):
        t = cpool.tile([128, WC], F16, tag=f"ax{k}", name=f"ax{k}_{hb}_{wck}")
        nc.vector.tensor_scalar_mul(t[:], sx2, val)
        ax[k] = t[:]
        t = cpool.tile([128, WC], F16, tag=f"by{k}", name=f"by{k}_{hb}_{wck}")
        nc.vector.tensor_scalar_mul(t[:], sy2, val)
        by[k] = t[:]
    asp = {}
    for ai in range(4):
        for bi in range(4):
            if ai == 0 and bi == 0:
                continue
            if ai == 0:
                asp[(ai, bi)] = by[bi]
            elif bi == 0:
                asp[(ai, bi)] = ax[ai]
            else:
                t = cpool.tile([128, WC], F16, tag=f"asp{ai}{bi}",
                               name=f"asp{ai}{bi}_{hb}_{wck}")
                nc.vector.tensor_add(t[:], ax[ai], by[bi])
                asp[(ai, bi)] = t[:]

    # ---- persistent PSUM accumulators: wsum + 3 out channels ----
    # center tap (w=1) folded into the start=True init matmuls.
    ones = cpool.tile([128, WC], F16, tag="ones", name=f"ones_{hb}_{wck}")
    nc.vector.memset(ones[:], 1.0)
    accW = papool.tile([128, WC], FP32, tag="accW", name=f"accW_{hb}_{wck}")
    accC = papool.tile([128, CO * WC], FP32, tag="accC",
                       name=f"accC_{hb}_{wck}")
    nc.tensor.matmul(out=accW[:], lhsT=ident[:], rhs=ones[:],
                     start=True, stop=False)
    for c in range(CO):
        nc.tensor.matmul(out=accC[:, c * WC : (c + 1) * WC], lhsT=ident[:],
                         rhs=Fc[:, c, :], start=True, stop=False)

    # ---- 48 off-center taps ----
    taps = TAP_SET if TAP_SET is not None else [
        (i, j) for i in range(KS) for j in range(KS) if (i, j) != (RAD, RAD)]
    n_taps = len(taps)
    for ti, (i, j) in enumerate(taps):   # oy = i - 3, ox = j - 3
        last = ti == n_taps - 1
        sh = f3d(i)[:, :, j + 1 : j + 1 + WC]  # shifted view [128, 8, WC]
        d = dpool.tile([128, CF * WC], F16, tag="d",
                       name=f"d_{hb}_{wck}_{i}_{j}")
        nc.vector.tensor_sub(
            d[:].rearrange("p (c x) -> p c x", x=WC), sh, Fc)
        d2 = d
        nc.scalar.activation(d2[:], d[:],
                             mybir.ActivationFunctionType.Square)
        rd = dpool.tile([128, CF * WC], F16, tag="rd",
                        name=f"rd_{hb}_{wck}_{i}_{j}")
        nc.vector.tensor_mul(rd[:], R[:], d2[:])
        rd3 = rd[:].rearrange("p (c x) -> p c x", x=WC)
        # PE: -(channel reduce + Asp) accumulated in a PSUM bank
        logw = plpool.tile([128, WC], FP32, tag="logw", bufs=4,
                           name=f"logw_{hb}_{wck}_{i}_{j}")
        for c in range(CF):
            nc.tensor.matmul(out=logw[:], lhsT=identN[:], rhs=rd3[:, c, :],
                             start=(c == 0), stop=False)
        nc.tensor.matmul(out=logw[:], lhsT=identN[:],
                         rhs=asp[(IDX4[j], IDX4[i])], start=False, stop=True)
        w_t = spool.tile([128, WC], F16, tag="w",
                         name=f"w_{hb}_{wck}_{i}_{j}")
        nc.scalar.activation(w_t[:], logw[:],
                             mybir.ActivationFunctionType.Exp)
        # numerator: t3 = w * [f0, f1, f2]
        t3 = spool.tile([128, CO * WC], F16, tag="t3", bufs=3,
                        name=f"t3_{hb}_{wck}_{i}_{j}")
        w_b = w_t[:].unsqueeze(1).broadcast_to([128, CO, WC])
        nc.vector.tensor_mul(
            t3[:].rearrange("p (c x) -> p c x", x=WC), w_b,
            f3d(i)[:, 0:CO, j + 1 : j + 1 + WC])
        nc.tensor.matmul(out=accW[:], lhsT=ident[:], rhs=w_t[:],
                         start=False, stop=last)
        for c in range(CO):
            nc.tensor.matmul(out=accC[:, c * WC : (c + 1) * WC], lhsT=ident[:],
                             rhs=t3[:, c * WC : (c + 1) * WC],
                             start=False, stop=last)

    # ---- out = acc / wsum ----
    rec = spool.tile([128, WC], FP32, tag="rec", bufs=1,
                     name=f"rec_{hb}_{wck}")
    nc.vector.reciprocal_approx_fast(rec[:], accW[:])
    out3 = spool.tile([128, CO * WC], FP32, tag="out3", bufs=1,
                      name=f"out3_{hb}_{wck}")
    for c in range(CO):
        nc.vector.tensor_mul(out3[:, c * WC : (c + 1) * WC], rec[:],
                             accC[:, c * WC : (c + 1) * WC])
    o3 = out3[:].rearrange("p (c x) -> p c x", x=WC)
    for c in range(CO):
        nc.sync.dma_start(out=y[c, r0 : r0 + 128, w0 : w0 + WC],
                          in_=o3[:, c, :])


def shard_inputs(input):
    """input [2,18,1024,1024] -> 8 per-core slabs [18, 262, 1030]."""
    input = np.asarray(input, dtype=np.float32)
    per_b = 4
    rows = H // per_b
    in_maps = []
    for core in range(8):
        b, q = divmod(core, per_b)
        r0 = q * rows
        slab = np.full((C_ALL, HIN, WIN), SENT, dtype=np.float32)
        s_lo = max(r0 - RAD, 0)
        s_hi = min(r0 + rows + RAD, H)
        slab[:, s_lo - (r0 - RAD) : s_hi - (r0 - RAD), LPAD : LPAD + W] = \
            input[b, :, s_lo:s_hi, :]
        in_maps.append({"x": np.ascontiguousarray(slab)})
    return in_maps


def assemble(results):
    out = np.empty((B, CO, H, W), dtype=np.float32)
    rows = H // 4
    for core in range(8):
        b, q = divmod(core, 4)
        out[b, :, q * rows : (q + 1) * rows, :] = results[core]["y"]
    return out


def kernel(input):
    from concourse.bass_utils import run_bass_kernel_spmd

    if "nc" not in _CACHED:
        _CACHED["nc"] = build_nc()
    in_maps = shard_inputs(input)
    res = run_bass_kernel_spmd(_CACHED["nc"], in_maps, list(range(8)))
    return assemble(res.results)
